# revision 36
# baseline (speedup 1.0000x reference)
"""Trainium2 Bass kernel: causal multi-head attention with interleaved RoPE.

Problem shapes (hardcoded): x [2, 2048, 1024], 16 heads of dk=64.
Sharding: 8 cores = 2 batches x 4 head-groups (4 heads each). Each core
computes its head-slice Q/K/V projections, RoPE, causal attention, and a
partial output through its Wo row-slice; the host sums the 4 partials per
batch and adds bo.

RoPE trick: attention scores are invariant to any permutation of the dk
axis applied to both Q and K, so the Wq/Wk columns are permuted on the host
into a "quadrant half-split" layout where each rotation pair partner sits
exactly 16 partitions away inside the same 32-partition quadrant. The DVE
stream_shuffle (a per-quadrant 32-way permute) then produces the swapped
operand, and RoPE becomes: rot = q * cosT + shuffle(q) * sinT with
host-precomputed tables (sinT carries the sign).

Schedule highlights:
- The two heads of a chunk run as row-tiled CONCURRENT score matmuls
  (PE tiles T0/T8: contract=64 on partitions 0:64 / 64:128).
- Attention groups are processed in PAIRS so the PE alternates between one
  64-mode stretch (4 score MMs) and one 128-mode stretch (4 PV MMs +
  projection/Wo fillers), halving tiling-mode-switch drains.
- PV for the odd head uses a zero-padded [128]-col V operand so its output
  lands on PSUM partitions 64:128 (denominator at row 0): the chunk's two
  normalized heads stack into one [128, 512] tile, making the Wo projection
  contract-128 (2 MMs instead of 4) and the softmax normalization a
  broadcast matmul + reciprocal_approx_fast + two aligned muls. Block 3's
  chunk-0 Wo half streams out early through `out2` (host adds it).
- Score/PV matmuls on causal-diagonal key tiles skip the fully-masked
  column prefix (partial-N matmuls; no prefix memsets).
- Dummy N=64 matmuls during the initial DMA wait warm the PE HAM clock
  gate so real matmuls start at 2.4 GHz; norm-pinned dummies keep it warm
  through the final normalize chain. Projection/V/Wo fillers are
  interleaved by kind so consecutive fillers never serialize on the
  2-buffer matmul-psum pool behind the RoPE chain's reads.
"""

import os
from collections import deque
from contextlib import ExitStack

import numpy as np
import ml_dtypes

import concourse.mybir as mybir
import concourse.tile as tile

B, S, D, H = 2, 2048, 1024, 16
DK = D // H  # 64
HG = 4  # heads per core
NCOLS = HG * DK  # 256 columns of the projection per core
THETA = 10000.0
SCALE = 1.0 / float(np.sqrt(DK))
N_CORES = 8

F32 = mybir.dt.float32
BF16 = mybir.dt.bfloat16

SB = 512            # sq block width
NSB = S // SB       # 4
NST = S // 128      # 16 key tiles / V tiles
NDC = D // 128      # 8 contraction chunks
WS = 1.0            # (fp8 experiment removed; keep host API stable)
# vaug per-chunk layout: [V_even(64) | one | pad(7) | one | zeros(63) | V_odd(64)]
P0W = 72            # parity-0 slice width (V 64 + one + pad to 16B alignment)
CW = P0W + 128      # per-chunk width (parity-1 slice is a full 128 cols)

BF = ml_dtypes.bfloat16


def to_bf16(a):
    return np.ascontiguousarray(np.asarray(a, dtype=np.float32)).astype(BF)


# ---------------------------------------------------------------------------
# host-side prep
# ---------------------------------------------------------------------------

def _rope_perm():
    """Within-head column permutation pi: new row r -> original dk index."""
    perm = np.empty(DK, dtype=np.int64)
    for r in range(DK):
        q, m = divmod(r, 32)
        if m < 16:
            perm[r] = 2 * (16 * q + m)
        else:
            perm[r] = 2 * (16 * q + m - 16) + 1
    return perm


_PERM = _rope_perm()
SHUF_MASK = list(range(16, 32)) + list(range(16))  # swap 16-halves per quadrant


def _causal_masks():
    """Band-local causal keep-mask [128, 2, 128]: mask[p, t, fl] = (fl >= p).
    The 128-wide triangular band of ANY diagonal tile reduces to this same
    pattern in band-local column coordinates; doubled along t so one multiply
    covers the fused even|odd head pair."""
    p_ = np.arange(128)[:, None, None]
    f_ = np.arange(128)[None, None, :]
    keep = (f_ >= p_) | (np.arange(2)[None, :, None] < 0)
    keep = np.broadcast_to(keep, (128, 2, 128))
    return to_bf16(keep.astype(np.float32))


_CAUSAL_MASKS = _causal_masks()


def _rope_tables(pos):
    """cosT/sinT [128, S] fp32 for the permuted layout. pos: [S] int."""
    inv_freq = (np.float32(THETA) ** (-(np.arange(0, DK, 2, dtype=np.float32) / np.float32(DK))))  # [32]
    ang = pos.astype(np.float32)[:, None] * inv_freq[None, :]  # [S, 32]
    cos = np.cos(ang)  # [S, 32]
    sin = np.sin(ang)
    cosT = np.empty((128, S), dtype=np.float32)
    sinT = np.empty((128, S), dtype=np.float32)
    for p in range(128):
        r = p % DK
        q, m = divmod(r, 32)
        if m < 16:
            i = 16 * q + m
            sgn = -1.0
        else:
            i = 16 * q + m - 16
            sgn = 1.0
        cosT[p] = cos[:, i]
        sinT[p] = np.float32(sgn) * sin[:, i]
    return cosT, sinT


def _ones2():
    """[65, 128] head-pair broadcast selector: row 0 -> partitions 0:64
    (even-head denominator), row 32 -> partitions 64:128 (odd); other rows
    zero. 65 partitions round up to a 128-row PE tile, so the denominator
    broadcast matmul stays in 128-mode (a 33-row operand would force a
    64x128 tiling-mode switch and drain the array)."""
    o = np.zeros((65, 128), dtype=np.float32)
    o[0, 0:DK] = 1.0
    o[32, DK:128] = 1.0
    return to_bf16(o)


def make_core_inputs(x, token_position, Wq, bq, Wk, bk, Wv, bv, Wo, bo):
    """Build the 8 per-core input maps."""
    x = np.asarray(x, dtype=np.float32)
    token_position = np.asarray(token_position)
    Wq, Wk, Wv, Wo = (np.asarray(w, dtype=np.float32) for w in (Wq, Wk, Wv, Wo))
    bq, bk, bv = (np.asarray(b_, dtype=np.float32) for b_ in (bq, bk, bv))

    in_maps = []
    tables = {}
    for c in range(N_CORES):
        b, hg = divmod(c, HG)
        heads = range(HG * hg, HG * hg + HG)
        # permuted q/k column indices for this core's heads
        cols_qk = np.concatenate([DK * h + _PERM for h in heads])
        cols_v = np.arange(NCOLS * hg, NCOLS * hg + NCOLS)
        if b not in tables:
            tables[b] = _rope_tables(np.asarray(token_position[b]))
        cosT, sinT = tables[b]
        wo_rows = Wo[cols_v, :]  # [256, 1024]
        in_maps.append({
            "xT": to_bf16(x[b].T),                              # [1024, 2048]
            "wq": to_bf16(Wq[:, cols_qk]),                      # [1024, 256]
            "wk": to_bf16(Wk[:, cols_qk]),
            "wv": to_bf16(Wv[:, cols_v]),
            # head pair (2c, 2c+1) stacked on the contract dim: [128, 2, 1024]
            "wo": to_bf16(wo_rows.reshape(2, 128, D).transpose(1, 0, 2)),
            "bq": to_bf16(bq[cols_qk][None, :]),                # [1, 256]
            "bk": to_bf16(bk[cols_qk][None, :]),
            "bv": to_bf16(bv[cols_v][None, :]),
            "ones_row": to_bf16(np.ones((1, SB), np.float32)),
            "onesc": to_bf16(np.ones((128, DK), np.float32)),
            "ones2": _ones2(),
            "maskd": _CAUSAL_MASKS,
            "cosT": to_bf16(cosT),
            "sinT": to_bf16(sinT),
        })
    return in_maps


# ---------------------------------------------------------------------------
# device program
# ---------------------------------------------------------------------------

def build_program(with_bias=False):
    from concourse import bacc, library_config
    nc = bacc.Bacc("TRN2", debug=False)

    xT = nc.declare_dram_parameter("xT", [D, S], BF16, isOutput=False).ap()
    wq = nc.declare_dram_parameter("wq", [D, NCOLS], BF16, isOutput=False).ap()
    wk = nc.declare_dram_parameter("wk", [D, NCOLS], BF16, isOutput=False).ap()
    wv = nc.declare_dram_parameter("wv", [D, NCOLS], BF16, isOutput=False).ap()
    wo = nc.declare_dram_parameter("wo", [128, 2, D], BF16, isOutput=False).ap()
    bq = nc.declare_dram_parameter("bq", [1, NCOLS], BF16, isOutput=False).ap()
    bk = nc.declare_dram_parameter("bk", [1, NCOLS], BF16, isOutput=False).ap()
    bv = nc.declare_dram_parameter("bv", [1, NCOLS], BF16, isOutput=False).ap()
    ones_row_d = nc.declare_dram_parameter("ones_row", [1, SB], BF16, isOutput=False).ap()
    onesc_d = nc.declare_dram_parameter("onesc", [128, DK], BF16, isOutput=False).ap()
    ones2_d = nc.declare_dram_parameter("ones2", [65, 128], BF16, isOutput=False).ap()
    maskd_d = nc.declare_dram_parameter("maskd", [128, 2, 128], BF16, isOutput=False).ap()
    cosT = nc.declare_dram_parameter("cosT", [128, S], BF16, isOutput=False).ap()
    sinT = nc.declare_dram_parameter("sinT", [128, S], BF16, isOutput=False).ap()
    out = nc.declare_dram_parameter("out", [S, D], BF16, isOutput=True).ap()
    out2 = nc.declare_dram_parameter("out2", [SB, D], BF16, isOutput=True).ap()
    debug_dump = os.environ.get("KERNEL_DEBUG_DUMP", "0") == "1"
    if debug_dump:
        dbg_qt = nc.declare_dram_parameter("dbg_qt", [128, SB], F32, isOutput=True).ap()
        dbg_kh = nc.declare_dram_parameter("dbg_kh", [128, SB], F32, isOutput=True).ap()
        dbg_va = nc.declare_dram_parameter("dbg_va", [128, 2 * CW], F32, isOutput=True).ap()
        dbg_ot = nc.declare_dram_parameter("dbg_ot", [NSB, 2, 128, SB], F32, isOutput=True).ap()

    with tile.TileContext(nc) as tc, ExitStack() as ctx:
        nc.gpsimd.load_library(library_config.proxy)
        const = ctx.enter_context(tc.tile_pool(name="const", bufs=1))
        sbig = ctx.enter_context(tc.tile_pool(name="sbig", bufs=1))
        rtmp = ctx.enter_context(tc.tile_pool(name="rtmp", bufs=3))
        epool = ctx.enter_context(tc.tile_pool(name="epool", bufs=4))
        npool = ctx.enter_context(tc.tile_pool(name="npool", bufs=2))
        opool = ctx.enter_context(tc.tile_pool(name="opool", bufs=3))
        sc_ps = ctx.enter_context(tc.tile_pool(name="sc_ps", bufs=2, space="PSUM"))
        pv_ps = ctx.enter_context(tc.tile_pool(name="pv_ps", bufs=2, space="PSUM"))
        mm_ps = ctx.enter_context(tc.tile_pool(name="mm_ps", bufs=2, space="PSUM"))

        # --- static SBUF tiles
        wq_sb = [const.tile([128, NCOLS], BF16, tag=f"wq{dc}", name=f"wq{dc}")
                 for dc in range(NDC)]
        wk_sb = [const.tile([128, NCOLS], BF16, tag=f"wk{dc}", name=f"wk{dc}")
                 for dc in range(NDC)]
        wv_sb = [const.tile([128, NCOLS], BF16, tag=f"wv{dc}", name=f"wv{dc}")
                 for dc in range(NDC)]
        cos_bf = const.tile([128, S], BF16, tag="cosb")
        sin_bf = const.tile([128, S], BF16, tag="sinb")
        wo_sb = const.tile([128, 2, D], BF16, tag="wo")
        onesc_sb = const.tile([128, DK], BF16, tag="onesc")
        ones2_sb = const.tile([65, 128], BF16, tag="ones2")
        mask_sb = const.tile([128, 2, 128], BF16, tag="maskd")
        if with_bias:
            bq_sb = const.tile([1, NCOLS], BF16, tag="bq")
            bk_sb = const.tile([1, NCOLS], BF16, tag="bk")
            bv_sb = const.tile([1, NCOLS], BF16, tag="bv")
            ones_row = const.tile([1, SB], BF16, tag="ones_row")
        xt = [[sbig.tile([128, SB], BF16, tag=f"xt{sb}_{dc}", name=f"xt{sb}_{dc}")
               for dc in range(NDC)] for sb in range(NSB)]
        # Q^T / K^T per (chunk, sq-block): chunk c holds heads {2c, 2c+1}
        # stacked on partitions (head 2c rows 0:64, head 2c+1 rows 64:128)
        qt = [[sbig.tile([128, SB], BF16, tag=f"qt{c}_{sb}", name=f"qt{c}_{sb}")
               for sb in range(NSB)] for c in range(2)]
        kth = [[sbig.tile([128, SB], BF16, tag=f"kh{c}_{sb}", name=f"kh{c}_{sb}")
                for sb in range(NSB)] for c in range(2)]
        # V augmented per key tile, per chunk [V_e|1|pad | 1|0*63|V_o] so the
        # parity-1 PV lands on psum partitions 64:128 and both PVs carry the
        # softmax denominator (rows 64 / 0)
        vaug = [sbig.tile([128, 2 * CW], BF16, tag=f"va{st}", name=f"va{st}")
                for st in range(NST)]
        # normalized O^T per (chunk, sq-block): the chunk's head pair stacked
        # on partitions, ready as a contract-128 Wo operand
        ot2 = [[sbig.tile([128, SB], BF16, tag=f"ot{c}_{j}", name=f"ot{c}_{j}")
                for j in range(NSB)] for c in range(2)]
        # denominator staging: [33, SB] with den_even at row 0, den_odd at
        # row 32; rows 1-31 zeroed once (they hit zero ones2 rows anyway but
        # must not be NaN)
        den2s = [sbig.tile([65, SB], BF16, tag=f"den{cc}", name=f"den{cc}")
                 for cc in range(2)]
        for cc in range(2):
            nc.vector.memset(den2s[cc][0:65, :], 0.0)
        # zero the parity-1 zero-block of every vaug tile once (static tiles)
        for st in range(NST):
            va = vaug[st][:].rearrange("p (c w) -> p c w", c=2)
            nc.vector.memset(va[:, :, P0W + 1:P0W + DK], 0.0)

        # --- DMAs, critical-path first: block-0 operands, then the rest
        nc.sync.dma_start(onesc_sb[:], onesc_d)
        for dc in range(NDC):
            nc.sync.dma_start(wq_sb[dc][:], wq[128 * dc:128 * dc + 128, :])
            nc.sync.dma_start(xt[0][dc][:], xT[128 * dc:128 * dc + 128, 0:SB])
        nc.sync.dma_start(cos_bf[:, 0:SB], cosT[:, 0:SB])
        nc.sync.dma_start(sin_bf[:, 0:SB], sinT[:, 0:SB])
        for dc in range(NDC):
            nc.sync.dma_start(wk_sb[dc][:], wk[128 * dc:128 * dc + 128, :])
        nc.sync.dma_start(mask_sb[:], maskd_d)
        for dc in range(NDC):
            nc.sync.dma_start(wv_sb[dc][:], wv[128 * dc:128 * dc + 128, :])
        for dc in range(NDC):
            nc.sync.dma_start(xt[1][dc][:], xT[128 * dc:128 * dc + 128, SB:2 * SB])
        nc.sync.dma_start(cos_bf[:, SB:2 * SB], cosT[:, SB:2 * SB])
        nc.sync.dma_start(sin_bf[:, SB:2 * SB], sinT[:, SB:2 * SB])
        nc.sync.dma_start(ones2_sb[:], ones2_d)
        nc.sync.dma_start(wo_sb[:], wo)
        if with_bias:
            nc.sync.dma_start(bq_sb[:], bq)
            nc.sync.dma_start(bk_sb[:], bk)
            nc.sync.dma_start(bv_sb[:], bv)
            nc.sync.dma_start(ones_row[:], ones_row_d)
        for sb in (2, 3):
            for dc in range(NDC):
                nc.sync.dma_start(xt[sb][dc][:],
                                  xT[128 * dc:128 * dc + 128, SB * sb:SB * sb + SB])
            nc.sync.dma_start(cos_bf[:, SB * sb:SB * sb + SB],
                              cosT[:, SB * sb:SB * sb + SB])
            nc.sync.dma_start(sin_bf[:, SB * sb:SB * sb + SB],
                              sinT[:, SB * sb:SB * sb + SB])

        # --- PE warmup: dummy matmuls fill the initial DMA wait so the HAM
        # clock gate reaches 8/8 before the first real matmul, and N=512
        # dummies keep it warm through the DMA-staggered first projections
        warm = mm_ps.tile([128, SB], F32, tag="mm", name="warm")
        for _ in range(56):
            nc.tensor.matmul(warm[0:DK, 0:DK], onesc_sb[:, 0:DK],
                             onesc_sb[:, 0:DK], start=True, stop=True)

        # --- emission helpers -------------------------------------------
        pending_add = deque()  # rope adds, deferred one chunk so the
        # gpsimd sin-mul latency never head-blocks the in-order DVE queue

        def emit_qk_chunk(sb, c, wname, pool=None):
            """Projection chunk c of Q or K for sq block sb, incl. RoPE."""
            w_sb = wq_sb if wname == "q" else wk_sb
            ss = slice(SB * sb, SB * sb + SB)
            ncol = slice(128 * c, 128 * c + 128)
            if pool == "sc":
                ps = sc_ps.tile([128, 2, SB], F32, tag="sc",
                                name="ps_qk")[:, 0, :]
            else:
                ps = mm_ps.tile([128, SB], F32, tag="mm", name="ps_qk")
            for dc in range(NDC):
                nc.tensor.matmul(ps[:], w_sb[dc][:, ncol], xt[sb][dc][:],
                                 start=(dc == 0),
                                 stop=(dc == NDC - 1 and not with_bias))
            if with_bias:
                b_sb = bq_sb if wname == "q" else bk_sb
                nc.tensor.matmul(ps[:], b_sb[0:1, ncol], ones_row[0:1, :],
                                 start=False, stop=True)
            # rope: dst = ps*cos + shuffle(ps)*sin
            t_cos = rtmp.tile([128, SB], F32, tag="rc", name="t_cos")
            nc.vector.tensor_mul(t_cos[:], ps[:], cos_bf[:, ss])
            t_shuf = rtmp.tile([128, SB], F32, tag="rs", name="t_shuf")
            nc.vector.stream_shuffle(t_shuf[:], ps[:], SHUF_MASK)
            t_sin = rtmp.tile([128, SB], F32, tag="rm", name="t_sin")
            nc.gpsimd.tensor_mul(t_sin[:], t_shuf[:], sin_bf[:, ss])
            dst = qt[c][sb] if wname == "q" else kth[c][sb]
            while pending_add:
                pending_add.popleft()()
            pending_add.append(
                lambda dst=dst, t_cos=t_cos, t_sin=t_sin:
                    nc.vector.tensor_add(dst[:], t_cos[:], t_sin[:]))

        def emit_v_st(sb, st4, pool=None):
            """V projection for one 128-seq tile, scattered into vaug."""
            st = 4 * sb + st4
            if pool == "sc":
                ps = sc_ps.tile([128, 2, SB], F32, tag="sc",
                                name="ps_v")[:, 0, :]
            else:
                ps = mm_ps.tile([128, SB], F32, tag="mm", name="ps_v")
            for dc in range(NDC):
                nc.tensor.matmul(ps[:, 0:NCOLS],
                                 xt[sb][dc][:, 128 * st4:128 * st4 + 128],
                                 wv_sb[dc][:],
                                 start=(dc == 0),
                                 stop=(dc == NDC - 1 and not with_bias))
            if with_bias:
                nc.tensor.matmul(ps[:, 0:NCOLS], ones_row[0:1, 0:128],
                                 bv_sb[0:1, :], start=False, stop=True)
            va = vaug[st][:].rearrange("p (c w) -> p c w", c=2)
            psv = ps[:, 0:NCOLS].rearrange("p (h k) -> p h k", h=HG)
            nc.vector.tensor_copy(va[:, :, 0:DK], psv[:, 0::2, :])
            nc.vector.tensor_copy(va[:, :, P0W + DK:P0W + 2 * DK],
                                  psv[:, 1::2, :])
            nc.vector.tensor_copy(va[:, :, DK], onesc_sb[:, 0:2])
            nc.vector.tensor_copy(va[:, :, P0W], onesc_sb[:, 0:2])
            while pending_add:
                pending_add.popleft()()

        def emit_wo(st, dc, copy_eng="v", ps=None, chunks=(0, 1), half=False):
            """Output projection for one (128-seq, 512-dmodel) tile. With
            half=True only the given chunk's head pair contributes and the
            partial goes to out2 (block-3 rows; the host adds it)."""
            jb = st // 4
            rq = slice(128 * (st % 4), 128 * (st % 4) + 128)
            cols = slice(SB * dc, SB * dc + SB)
            if ps is None:
                ps = mm_ps.tile([128, SB], F32, tag="mm", name="ps_wo")
            for ci, c in enumerate(chunks):
                nc.tensor.matmul(ps[:], ot2[c][jb][:, rq], wo_sb[:, c, cols],
                                 start=(ci == 0), stop=(ci == len(chunks) - 1))
            o_sb = opool.tile([128, SB], BF16, tag="osb", name="o_sb")
            if copy_eng == "s":
                nc.scalar.copy(o_sb[:], ps[:])
            else:
                nc.vector.tensor_copy(o_sb[:], ps[:])
            if half:
                nc.sync.dma_start(out2[128 * (st % 4):128 * (st % 4) + 128, cols],
                                  o_sb[:])
            else:
                nc.sync.dma_start(out[128 * st:128 * st + 128, cols], o_sb[:])

        # --- projections for block 0 chunk 0 (pair 1's chunks become
        # the first fillers inside pair 0's attention)
        emit_qk_chunk(0, 0, "q")
        emit_qk_chunk(0, 0, "k")
        for st4 in range(4):
            emit_v_st(0, st4)

        # --- main interleaved stream ------------------------------------
        seq = os.environ.get("KERNEL_SEQ", "0") == "1"
        if seq:
            emit_qk_chunk(0, 1, "q")
            emit_qk_chunk(0, 1, "k")
            for nb in range(1, NSB):
                emit_qk_chunk(nb, 0, "q")
                emit_qk_chunk(nb, 0, "k")
                emit_qk_chunk(nb, 1, "q")
                emit_qk_chunk(nb, 1, "k")
                for st4 in range(4):
                    emit_v_st(nb, st4)
        fillers = deque()
        if not seq:
            fillers.append(lambda: emit_qk_chunk(0, 1, "q"))
            fillers.append(lambda: emit_qk_chunk(0, 1, "k"))
        pending_norm = deque()  # closures, flushed in the next 128-mode stretch
        prev_pair = None  # (emit_pv, [(g, e), (g, e)], tail?) one pair behind
        for j in range(NSB):
            if not seq and j < NSB - 1:
                nb = j + 1
                # interleave projection kinds: consecutive qk chunks would
                # serialize on the 2-buffer mm psum pool behind the RoPE
                # chain's DVE reads; a V tile between them breaks the chain
                fillers.append(lambda nb=nb: emit_qk_chunk(nb, 0, "q"))
                fillers.append(lambda nb=nb, st4=0: emit_v_st(nb, st4))
                fillers.append(lambda nb=nb: emit_qk_chunk(nb, 0, "k"))
                fillers.append(lambda nb=nb, st4=1: emit_v_st(nb, st4))
                fillers.append(lambda nb=nb: emit_qk_chunk(nb, 1, "q"))
                fillers.append(lambda nb=nb, st4=2: emit_v_st(nb, st4))
                fillers.append(lambda nb=nb: emit_qk_chunk(nb, 1, "k"))
                fillers.append(lambda nb=nb, st4=3: emit_v_st(nb, st4))
            wo_blocks = {1: [0], 3: [1, 2]}
            if not seq:
                for jb in wo_blocks.get(j, []):
                    for st in range(4 * jb, 4 * jb + 4):
                        for dc in range(2):
                            fillers.append(
                                lambda st=st, dc=dc: emit_wo(st, dc))

            # Bresenham-spread the filler pops across the whole block so
            # late steps (right before the next block / the tail) are not
            # left dry by front-loaded FIFO popping
            navail = len(fillers) + (8 if (j == 3 and not seq) else 0)
            steps_total = 2 * 2 * (j + 1)  # pair-iterations in this block
            fill_state = [0, 0]  # steps seen, pops done

            # Head-pair interleave: the even and odd head of a chunk run as
            # row-tiled concurrent score matmuls; groups are processed in
            # pairs so 64-mode (scores) and 128-mode (PV + fillers) stretches
            # alternate only once per pair. PV runs one pair behind, and the
            # lag carries ACROSS chunk/block boundaries so the PE queue head
            # is never blocked on the Scalar exp backlog at a boundary.
            for c in range(2):
                while pending_add:  # qt/kth of this block must be complete
                    pending_add.popleft()()
                pvs = [pv_ps.tile([128, SB], F32, tag="pv", name="pv")
                       for _ in range(2)]
                ngrp = 4 * (j + 1)
                npairs = ngrp // 2

                def emit_sc(i, j=j, c=c):
                    # both heads' score tiles fused in one 2-bank psum tile
                    # so a single exp (and mask) covers the step. For tiles
                    # on the causal diagonal (d >= 0) sq columns < 128*d are
                    # fully masked: skip them in the matmul and exp, and
                    # mask-multiply only the 128-wide triangular band (the
                    # band-local mask is the same for every diagonal).
                    d = i - 4 * j
                    c0 = 128 * max(d, 0)
                    sc = sc_ps.tile([128, 2, SB], F32, tag="sc", name="sc")
                    for parity in range(2):
                        rows = slice(DK * parity, DK * parity + DK)
                        nc.tensor.matmul(
                            sc[:, parity, c0:],
                            kth[c][i // 4][rows, 128 * (i % 4):128 * (i % 4) + 128],
                            qt[c][j][rows, c0:],
                            start=True, stop=True)
                    e = epool.tile([128, 2, SB], BF16, tag="e", name="e")
                    nc.scalar.activation(e[:, :, c0:], sc[:, :, c0:],
                                         mybir.ActivationFunctionType.Exp,
                                         scale=SCALE)
                    if d >= 0:
                        band = slice(c0, c0 + 128)
                        nc.vector.tensor_mul(e[:, :, band], e[:, :, band],
                                             mask_sb[:])
                    return e

                def make_pv(j, c, pvs, ngrp):
                    def emit_pv(i, e):
                        d = i - 4 * j
                        c0 = 128 * max(d, 0)
                        va = vaug[i][:].rearrange("p (cc w) -> p cc w", cc=2)
                        st_ = (i == 0)
                        sp_ = (i == ngrp - 1)
                        nc.tensor.matmul(pvs[0][0:DK + 1, c0:],
                                         va[:, c, 0:DK + 1], e[:, 0, c0:],
                                         start=st_, stop=sp_)
                        nc.tensor.matmul(pvs[1][:, c0:],
                                         va[:, c, P0W:P0W + 128], e[:, 1, c0:],
                                         start=st_, stop=sp_)
                    return emit_pv

                def emit_norm(pv0, pv1, den2b, cc, jj):
                    bc2 = mm_ps.tile([128, SB], F32, tag="mm", name="bc2")
                    nc.tensor.matmul(bc2[:], ones2_sb[:], den2b[:],
                                     start=True, stop=True)
                    rec = npool.tile([128, SB], F32, tag="rec", name="rec")
                    nc.vector.reciprocal_approx_fast(rec[:], bc2[:])
                    nc.vector.tensor_mul(ot2[cc][jj][0:DK, :],
                                         pv0[0:DK, :], rec[0:DK, :])
                    nc.vector.tensor_mul(ot2[cc][jj][DK:128, :],
                                         pv1[DK:128, :], rec[DK:128, :])

                def make_tail(j, c, pvs):
                    def tail():
                        pvc = npool.tile([128, SB], F32, tag="pvc",
                                         name="pvc")
                        # stage the two denominator rows (bf16) for the
                        # deferred normalize. DVE copies keep the Scalar
                        # queue clear for the next chunk's exps; the final
                        # chunk uses Scalar so the den and pv stagings run
                        # on parallel queues at the tail.
                        if j == NSB - 1 and c == 1:
                            # one row per engine: halves the den-staging
                            # latency that the warm-keepers wait on, keeping
                            # the idle below the HAM re-throttle window
                            nc.scalar.copy(den2s[c][0:1, :], pvs[0][DK:DK + 1, :])
                            nc.vector.tensor_copy(den2s[c][32:33, :],
                                                  pvs[1][0:1, :])
                        else:
                            nc.vector.tensor_copy(den2s[c][0:1, :],
                                                  pvs[0][DK:DK + 1, :])
                            nc.vector.tensor_copy(den2s[c][32:33, :],
                                                  pvs[1][0:1, :])
                        nc.vector.tensor_copy(pvc[0:DK, :], pvs[0][0:DK, :])
                        nc.vector.tensor_copy(pvc[DK:128, :],
                                              pvs[1][DK:128, :])
                        pending_norm.append(
                            lambda pvc=pvc, den2b=den2s[c], cc=c, jj=j:
                                emit_norm(pvc, pvc, den2b, cc, jj))
                    return tail

                emit_pv = make_pv(j, c, pvs, ngrp)
                for gp in range(npairs):
                    g0, g1 = 2 * gp, 2 * gp + 1
                    cur = (emit_pv,
                           [(g0, emit_sc(g0)), (g1, emit_sc(g1))],
                           make_tail(j, c, pvs) if gp == npairs - 1 else None)
                    if gp == 0 and prev_pair is not None:
                        # boundary: the carried pair's exps were enqueued a
                        # full iteration ago; emitting its PVs (and the den
                        # staging) before the fillers puts the den copies
                        # ahead of the fillers' DVE work
                        pfn, pairs, ptail = prev_pair
                        for g, e in pairs:
                            pfn(g, e)
                        if ptail is not None:
                            ptail()
                        prev_pair = None
                    if j == 3 and c == 1 and gp == 1 and not seq:
                        # block-3 chunk-0 ot2 is normalized now: its Wo half
                        # runs as fillers; the host adds the out2 partial
                        for st_ in range(12, 16):
                            for dc_ in range(2):
                                fillers.append(
                                    lambda st_=st_, dc_=dc_: emit_wo(
                                        st_, dc_, chunks=(0,), half=True))
                    fill_state[0] += 1
                    while (fillers and fill_state[1] * steps_total <
                           fill_state[0] * navail):
                        fillers.popleft()()
                        fill_state[1] += 1
                    while pending_add:
                        pending_add.popleft()()
                    if prev_pair is not None:
                        pfn, pairs, ptail = prev_pair
                        for g, e in pairs:
                            pfn(g, e)
                        if ptail is not None:
                            ptail()
                    prev_pair = cur
                    while pending_norm:
                        pending_norm.popleft()()

        # --- tail: flush the carried pair, then the last normalize + Wo for
        # block 3; dummy matmuls keep the PE warm (and busy) while the final
        # normalize chain drains
        pfn, pairs, ptail = prev_pair
        for g, e in pairs:
            pfn(g, e)
        ptail()
        while pending_norm:
            pending_norm.popleft()()
        # warm-keepers: pinned behind the final den staging (rhs dep) so the
        # scheduler cannot hoist them; they keep the PE at 2.4 GHz while the
        # final recip/mul chain drains
        warm2 = mm_ps.tile([128, SB], F32, tag="mm", name="warm2")
        for _ in range(12):
            nc.tensor.matmul(warm2[:], ones2_sb[:],
                             den2s[1][:], start=True, stop=True)
        while fillers:
            fillers.popleft()()
        if seq:
            for st in range(0, 12):
                for dc in range(2):
                    emit_wo(st, dc)
            for st in range(12, 16):
                for dc in range(2):
                    emit_wo(st, dc, chunks=(0,), half=True)
        engs = ["v", "s"]
        for st in range(4 * (NSB - 1), 4 * NSB):
            # the sc psum pool is free at the tail: both 512-wide halves of a
            # seq tile go into one 2-bank sc tile, evacuated by a single
            # [128, 1024] copy + one full-row DMA (the banks are contiguous,
            # same pattern the exp reads); only the c1 head pair remains
            # (c0 went to out2 as fillers)
            sct = sc_ps.tile([128, 2, SB], F32, tag="sc", name="wops")
            rq = slice(128 * (st % 4), 128 * (st % 4) + 128)
            for dc in range(2):
                cols = slice(SB * dc, SB * dc + SB)
                nc.tensor.matmul(sct[:, dc, :], ot2[1][NSB - 1][:, rq],
                                 wo_sb[:, 1, cols], start=True, stop=True)
            o_sb = opool.tile([128, 2 * SB], BF16, tag="osb2", name="o_sb2")
            if engs[st % 2] == "s":
                nc.scalar.copy(o_sb[:], sct[:])
            else:
                nc.vector.tensor_copy(o_sb[:], sct[:])
            nc.sync.dma_start(out[128 * st:128 * st + 128, :], o_sb[:])

        if debug_dump:
            dq = opool.tile([128, SB], F32, tag="dbg", name="dq")
            nc.vector.tensor_copy(dq[:], qt[0][1][:])
            nc.sync.dma_start(dbg_qt, dq[:])
            dk_ = opool.tile([128, SB], F32, tag="dbg", name="dk_")
            nc.vector.tensor_copy(dk_[:], kth[0][1][:])
            nc.sync.dma_start(dbg_kh, dk_[:])
            dv = opool.tile([128, 2 * CW], F32, tag="dbgv", name="dv")
            nc.vector.tensor_copy(dv[:], vaug[4][:])
            nc.sync.dma_start(dbg_va, dv[:])
            for jj in range(NSB):
                for cc in range(2):
                    do = opool.tile([128, SB], F32, tag="dbg", name="do")
                    nc.vector.tensor_copy(do[:], ot2[cc][jj][:])
                    nc.sync.dma_start(dbg_ot[jj, cc], do[:])

    nc.compile()
    return nc


_CACHED_NC = {}


def _get_program(with_bias=False):
    if with_bias not in _CACHED_NC:
        _CACHED_NC[with_bias] = build_program(with_bias=with_bias)
    return _CACHED_NC[with_bias]


# ---------------------------------------------------------------------------
# entry point
# ---------------------------------------------------------------------------

def kernel(x, token_position, Wq, bq, Wk, bk, Wv, bv, Wo, bo, _results=None):
    from concourse.bass_utils import run_bass_kernel_spmd

    in_maps = make_core_inputs(x, token_position, Wq, bq, Wk, bk, Wv, bv, Wo, bo)
    if _results is None:
        with_bias = any(float(np.abs(np.asarray(v)).max()) != 0.0
                        for v in (bq, bk, bv))
        nc = _get_program(with_bias=with_bias)
        res = run_bass_kernel_spmd(nc, in_maps, list(range(N_CORES)))
        _results = [res.results[i] for i in range(N_CORES)]
    bo = np.asarray(bo, dtype=np.float32)
    out = np.empty((B, S, D), dtype=np.float32)
    for b in range(B):
        acc = np.zeros((S, D), dtype=np.float32)
        for hg in range(HG):
            r = _results[HG * b + hg]
            acc = acc + np.asarray(r["out"], dtype=np.float32)
            acc[S - SB:] += np.asarray(r["out2"], dtype=np.float32)
        out[b] = acc + bo[None, :]
    return out


# revision 37
# speedup vs baseline: 1.0299x; 1.0299x over previous
"""Trainium2 Bass kernel: causal multi-head attention with interleaved RoPE.

Problem shapes (hardcoded): x [2, 2048, 1024], 16 heads of dk=64.
Sharding: 8 cores = 2 batches x 4 head-groups (4 heads each). Each core
computes its head-slice Q/K/V projections, RoPE, causal attention, and a
partial output through its Wo row-slice; the host sums the 4 partials per
batch and adds bo.

RoPE trick: attention scores are invariant to any permutation of the dk
axis applied to both Q and K, so the Wq/Wk columns are permuted on the host
into a "quadrant half-split" layout where each rotation pair partner sits
exactly 16 partitions away inside the same 32-partition quadrant. The DVE
stream_shuffle (a per-quadrant 32-way permute) then produces the swapped
operand, and RoPE becomes: rot = q * cosT + shuffle(q) * sinT with
host-precomputed tables (sinT carries the sign).

Schedule highlights:
- The two heads of a chunk run as row-tiled CONCURRENT score matmuls
  (PE tiles T0/T8: contract=64 on partitions 0:64 / 64:128).
- Attention groups are processed in PAIRS so the PE alternates between one
  64-mode stretch (4 score MMs) and one 128-mode stretch (4 PV MMs +
  projection/Wo fillers), halving tiling-mode-switch drains.
- PV for the odd head uses a zero-padded [128]-col V operand so its output
  lands on PSUM partitions 64:128 (denominator at row 0): the chunk's two
  normalized heads stack into one [128, 512] tile, making the Wo projection
  contract-128 (2 MMs instead of 4) and the softmax normalization a
  broadcast matmul + reciprocal_approx_fast + two aligned muls. Block 3's
  chunk-0 Wo half streams out early through `out2` (host adds it).
- Score/PV matmuls on causal-diagonal key tiles skip the fully-masked
  column prefix (partial-N matmuls; no prefix memsets).
- Dummy N=64 matmuls during the initial DMA wait warm the PE HAM clock
  gate so real matmuls start at 2.4 GHz; norm-pinned dummies keep it warm
  through the final normalize chain. Projection/V/Wo fillers are
  interleaved by kind so consecutive fillers never serialize on the
  2-buffer matmul-psum pool behind the RoPE chain's reads.
"""

import os
from collections import deque
from contextlib import ExitStack

import numpy as np
import ml_dtypes

import concourse.mybir as mybir
import concourse.tile as tile

B, S, D, H = 2, 2048, 1024, 16
DK = D // H  # 64
HG = 4  # heads per core
NCOLS = HG * DK  # 256 columns of the projection per core
THETA = 10000.0
SCALE = 1.0 / float(np.sqrt(DK))
N_CORES = 8

F32 = mybir.dt.float32
BF16 = mybir.dt.bfloat16

SB = 512            # sq block width
NSB = S // SB       # 4
NST = S // 128      # 16 key tiles / V tiles
NDC = D // 128      # 8 contraction chunks
WS = 1.0            # (fp8 experiment removed; keep host API stable)
# vaug per-chunk layout: [V_even(64) | one | pad(7) | one | zeros(63) | V_odd(64)]
P0W = 72            # parity-0 slice width (V 64 + one + pad to 16B alignment)
CW = P0W + 128      # per-chunk width (parity-1 slice is a full 128 cols)

BF = ml_dtypes.bfloat16


def to_bf16(a):
    return np.ascontiguousarray(np.asarray(a, dtype=np.float32)).astype(BF)


# ---------------------------------------------------------------------------
# host-side prep
# ---------------------------------------------------------------------------

def _rope_perm():
    """Within-head column permutation pi: new row r -> original dk index."""
    perm = np.empty(DK, dtype=np.int64)
    for r in range(DK):
        q, m = divmod(r, 32)
        if m < 16:
            perm[r] = 2 * (16 * q + m)
        else:
            perm[r] = 2 * (16 * q + m - 16) + 1
    return perm


_PERM = _rope_perm()
SHUF_MASK = list(range(16, 32)) + list(range(16))  # swap 16-halves per quadrant


def _causal_masks():
    """Band-local causal keep-mask [128, 2, 128]: mask[p, t, fl] = (fl >= p).
    The 128-wide triangular band of ANY diagonal tile reduces to this same
    pattern in band-local column coordinates; doubled along t so one multiply
    covers the fused even|odd head pair."""
    p_ = np.arange(128)[:, None, None]
    f_ = np.arange(128)[None, None, :]
    keep = (f_ >= p_) | (np.arange(2)[None, :, None] < 0)
    keep = np.broadcast_to(keep, (128, 2, 128))
    return to_bf16(keep.astype(np.float32))


_CAUSAL_MASKS = _causal_masks()


def _rope_tables(pos):
    """cosT/sinT [128, S] fp32 for the permuted layout. pos: [S] int."""
    inv_freq = (np.float32(THETA) ** (-(np.arange(0, DK, 2, dtype=np.float32) / np.float32(DK))))  # [32]
    ang = pos.astype(np.float32)[:, None] * inv_freq[None, :]  # [S, 32]
    cos = np.cos(ang)  # [S, 32]
    sin = np.sin(ang)
    cosT = np.empty((128, S), dtype=np.float32)
    sinT = np.empty((128, S), dtype=np.float32)
    for p in range(128):
        r = p % DK
        q, m = divmod(r, 32)
        if m < 16:
            i = 16 * q + m
            sgn = -1.0
        else:
            i = 16 * q + m - 16
            sgn = 1.0
        cosT[p] = cos[:, i]
        sinT[p] = np.float32(sgn) * sin[:, i]
    return cosT, sinT


def _ones2():
    """[65, 128] head-pair broadcast selector: row 0 -> partitions 0:64
    (even-head denominator), row 32 -> partitions 64:128 (odd); other rows
    zero. 65 partitions round up to a 128-row PE tile, so the denominator
    broadcast matmul stays in 128-mode (a 33-row operand would force a
    64x128 tiling-mode switch and drain the array)."""
    o = np.zeros((65, 128), dtype=np.float32)
    o[0, 0:DK] = 1.0
    o[32, DK:128] = 1.0
    return to_bf16(o)


def make_core_inputs(x, token_position, Wq, bq, Wk, bk, Wv, bv, Wo, bo):
    """Build the 8 per-core input maps."""
    x = np.asarray(x, dtype=np.float32)
    token_position = np.asarray(token_position)
    Wq, Wk, Wv, Wo = (np.asarray(w, dtype=np.float32) for w in (Wq, Wk, Wv, Wo))
    bq, bk, bv = (np.asarray(b_, dtype=np.float32) for b_ in (bq, bk, bv))

    in_maps = []
    tables = {}
    for c in range(N_CORES):
        b, hg = divmod(c, HG)
        heads = range(HG * hg, HG * hg + HG)
        # permuted q/k column indices for this core's heads
        cols_qk = np.concatenate([DK * h + _PERM for h in heads])
        cols_v = np.arange(NCOLS * hg, NCOLS * hg + NCOLS)
        if b not in tables:
            tables[b] = _rope_tables(np.asarray(token_position[b]))
        cosT, sinT = tables[b]
        wo_rows = Wo[cols_v, :]  # [256, 1024]
        in_maps.append({
            "xT": to_bf16(x[b].T),                              # [1024, 2048]
            "wq": to_bf16(Wq[:, cols_qk]),                      # [1024, 256]
            "wk": to_bf16(Wk[:, cols_qk]),
            "wv": to_bf16(Wv[:, cols_v]),
            # head pair (2c, 2c+1) stacked on the contract dim: [128, 2, 1024]
            "wo": to_bf16(wo_rows.reshape(2, 128, D).transpose(1, 0, 2)),
            "bq": to_bf16(bq[cols_qk][None, :]),                # [1, 256]
            "bk": to_bf16(bk[cols_qk][None, :]),
            "bv": to_bf16(bv[cols_v][None, :]),
            "ones_row": to_bf16(np.ones((1, SB), np.float32)),
            "onesc": to_bf16(np.ones((128, DK), np.float32)),
            "ones2": _ones2(),
            "maskd": _CAUSAL_MASKS,
            "cosT": to_bf16(cosT),
            "sinT": to_bf16(sinT),
        })
    return in_maps


# ---------------------------------------------------------------------------
# device program
# ---------------------------------------------------------------------------

def build_program(with_bias=False):
    from concourse import bacc, library_config
    nc = bacc.Bacc("TRN2", debug=False)

    xT = nc.declare_dram_parameter("xT", [D, S], BF16, isOutput=False).ap()
    wq = nc.declare_dram_parameter("wq", [D, NCOLS], BF16, isOutput=False).ap()
    wk = nc.declare_dram_parameter("wk", [D, NCOLS], BF16, isOutput=False).ap()
    wv = nc.declare_dram_parameter("wv", [D, NCOLS], BF16, isOutput=False).ap()
    wo = nc.declare_dram_parameter("wo", [128, 2, D], BF16, isOutput=False).ap()
    bq = nc.declare_dram_parameter("bq", [1, NCOLS], BF16, isOutput=False).ap()
    bk = nc.declare_dram_parameter("bk", [1, NCOLS], BF16, isOutput=False).ap()
    bv = nc.declare_dram_parameter("bv", [1, NCOLS], BF16, isOutput=False).ap()
    ones_row_d = nc.declare_dram_parameter("ones_row", [1, SB], BF16, isOutput=False).ap()
    onesc_d = nc.declare_dram_parameter("onesc", [128, DK], BF16, isOutput=False).ap()
    ones2_d = nc.declare_dram_parameter("ones2", [65, 128], BF16, isOutput=False).ap()
    maskd_d = nc.declare_dram_parameter("maskd", [128, 2, 128], BF16, isOutput=False).ap()
    cosT = nc.declare_dram_parameter("cosT", [128, S], BF16, isOutput=False).ap()
    sinT = nc.declare_dram_parameter("sinT", [128, S], BF16, isOutput=False).ap()
    out = nc.declare_dram_parameter("out", [S, D], BF16, isOutput=True).ap()
    out2 = nc.declare_dram_parameter("out2", [SB, D], BF16, isOutput=True).ap()
    debug_dump = os.environ.get("KERNEL_DEBUG_DUMP", "0") == "1"
    if debug_dump:
        dbg_qt = nc.declare_dram_parameter("dbg_qt", [128, SB], F32, isOutput=True).ap()
        dbg_kh = nc.declare_dram_parameter("dbg_kh", [128, SB], F32, isOutput=True).ap()
        dbg_va = nc.declare_dram_parameter("dbg_va", [128, 2 * CW], F32, isOutput=True).ap()
        dbg_ot = nc.declare_dram_parameter("dbg_ot", [NSB, 2, 128, SB], F32, isOutput=True).ap()

    with tile.TileContext(nc) as tc, ExitStack() as ctx:
        nc.gpsimd.load_library(library_config.proxy)
        const = ctx.enter_context(tc.tile_pool(name="const", bufs=1))
        sbig = ctx.enter_context(tc.tile_pool(name="sbig", bufs=1))
        rtmp = ctx.enter_context(tc.tile_pool(name="rtmp", bufs=3))
        epool = ctx.enter_context(tc.tile_pool(name="epool", bufs=4))
        npool = ctx.enter_context(tc.tile_pool(name="npool", bufs=2))
        opool = ctx.enter_context(tc.tile_pool(name="opool", bufs=3))
        sc_ps = ctx.enter_context(tc.tile_pool(name="sc_ps", bufs=2, space="PSUM"))
        pv_ps = ctx.enter_context(tc.tile_pool(name="pv_ps", bufs=2, space="PSUM"))
        mm_ps = ctx.enter_context(tc.tile_pool(name="mm_ps", bufs=2, space="PSUM"))

        # --- static SBUF tiles
        wq_sb = [const.tile([128, NCOLS], BF16, tag=f"wq{dc}", name=f"wq{dc}")
                 for dc in range(NDC)]
        wk_sb = [const.tile([128, NCOLS], BF16, tag=f"wk{dc}", name=f"wk{dc}")
                 for dc in range(NDC)]
        wv_sb = [const.tile([128, NCOLS], BF16, tag=f"wv{dc}", name=f"wv{dc}")
                 for dc in range(NDC)]
        cos_bf = const.tile([128, S], BF16, tag="cosb")
        sin_bf = const.tile([128, S], BF16, tag="sinb")
        wo_sb = const.tile([128, 2, D], BF16, tag="wo")
        onesc_sb = const.tile([128, DK], BF16, tag="onesc")
        ones2_sb = const.tile([65, 128], BF16, tag="ones2")
        mask_sb = const.tile([128, 2, 128], BF16, tag="maskd")
        if with_bias:
            bq_sb = const.tile([1, NCOLS], BF16, tag="bq")
            bk_sb = const.tile([1, NCOLS], BF16, tag="bk")
            bv_sb = const.tile([1, NCOLS], BF16, tag="bv")
            ones_row = const.tile([1, SB], BF16, tag="ones_row")
        xt = [[sbig.tile([128, SB], BF16, tag=f"xt{sb}_{dc}", name=f"xt{sb}_{dc}")
               for dc in range(NDC)] for sb in range(NSB)]
        # Q^T / K^T per (chunk, sq-block): chunk c holds heads {2c, 2c+1}
        # stacked on partitions (head 2c rows 0:64, head 2c+1 rows 64:128)
        qt = [[sbig.tile([128, SB], BF16, tag=f"qt{c}_{sb}", name=f"qt{c}_{sb}")
               for sb in range(NSB)] for c in range(2)]
        kth = [[sbig.tile([128, SB], BF16, tag=f"kh{c}_{sb}", name=f"kh{c}_{sb}")
                for sb in range(NSB)] for c in range(2)]
        # V augmented per key tile, per chunk [V_e|1|pad | 1|0*63|V_o] so the
        # parity-1 PV lands on psum partitions 64:128 and both PVs carry the
        # softmax denominator (rows 64 / 0)
        vaug = [sbig.tile([128, 2 * CW], BF16, tag=f"va{st}", name=f"va{st}")
                for st in range(NST)]
        # normalized O^T per (chunk, sq-block): the chunk's head pair stacked
        # on partitions, ready as a contract-128 Wo operand
        ot2 = [[sbig.tile([128, SB], BF16, tag=f"ot{c}_{j}", name=f"ot{c}_{j}")
                for j in range(NSB)] for c in range(2)]
        # denominator staging: [33, SB] with den_even at row 0, den_odd at
        # row 32; rows 1-31 zeroed once (they hit zero ones2 rows anyway but
        # must not be NaN)
        den2s = [sbig.tile([65, SB], BF16, tag=f"den{cc}", name=f"den{cc}")
                 for cc in range(2)]
        for cc in range(2):
            nc.vector.memset(den2s[cc][0:65, :], 0.0)
        # zero the parity-1 zero-block of every vaug tile once (static tiles)
        for st in range(NST):
            va = vaug[st][:].rearrange("p (c w) -> p c w", c=2)
            nc.vector.memset(va[:, :, P0W + 1:P0W + DK], 0.0)

        # --- DMAs, critical-path first: block-0 operands, then the rest
        nc.sync.dma_start(onesc_sb[:], onesc_d)
        for dc in range(NDC):
            nc.sync.dma_start(wq_sb[dc][:], wq[128 * dc:128 * dc + 128, :])
            nc.sync.dma_start(xt[0][dc][:], xT[128 * dc:128 * dc + 128, 0:SB])
        nc.sync.dma_start(cos_bf[:, 0:SB], cosT[:, 0:SB])
        nc.sync.dma_start(sin_bf[:, 0:SB], sinT[:, 0:SB])
        for dc in range(NDC):
            nc.sync.dma_start(wk_sb[dc][:], wk[128 * dc:128 * dc + 128, :])
        nc.sync.dma_start(mask_sb[:], maskd_d)
        for dc in range(NDC):
            nc.sync.dma_start(wv_sb[dc][:], wv[128 * dc:128 * dc + 128, :])
        for dc in range(NDC):
            nc.sync.dma_start(xt[1][dc][:], xT[128 * dc:128 * dc + 128, SB:2 * SB])
        nc.sync.dma_start(cos_bf[:, SB:2 * SB], cosT[:, SB:2 * SB])
        nc.sync.dma_start(sin_bf[:, SB:2 * SB], sinT[:, SB:2 * SB])
        nc.sync.dma_start(ones2_sb[:], ones2_d)
        nc.sync.dma_start(wo_sb[:], wo)
        if with_bias:
            nc.sync.dma_start(bq_sb[:], bq)
            nc.sync.dma_start(bk_sb[:], bk)
            nc.sync.dma_start(bv_sb[:], bv)
            nc.sync.dma_start(ones_row[:], ones_row_d)
        for sb in (2, 3):
            for dc in range(NDC):
                nc.sync.dma_start(xt[sb][dc][:],
                                  xT[128 * dc:128 * dc + 128, SB * sb:SB * sb + SB])
            nc.sync.dma_start(cos_bf[:, SB * sb:SB * sb + SB],
                              cosT[:, SB * sb:SB * sb + SB])
            nc.sync.dma_start(sin_bf[:, SB * sb:SB * sb + SB],
                              sinT[:, SB * sb:SB * sb + SB])

        # --- PE warmup: dummy matmuls fill the initial DMA wait so the HAM
        # clock gate reaches 8/8 before the first real matmul, and N=512
        # dummies keep it warm through the DMA-staggered first projections
        warm = mm_ps.tile([128, SB], F32, tag="mm", name="warm")
        for _ in range(56):
            nc.tensor.matmul(warm[0:DK, 0:DK], onesc_sb[:, 0:DK],
                             onesc_sb[:, 0:DK], start=True, stop=True)

        # --- emission helpers -------------------------------------------
        pending_add = deque()  # rope adds, deferred one chunk so the
        # gpsimd sin-mul latency never head-blocks the in-order DVE queue

        def emit_qk_chunk(sb, c, wname, pool=None):
            """Projection chunk c of Q or K for sq block sb, incl. RoPE."""
            w_sb = wq_sb if wname == "q" else wk_sb
            ss = slice(SB * sb, SB * sb + SB)
            ncol = slice(128 * c, 128 * c + 128)
            if pool == "sc":
                ps = sc_ps.tile([128, 2, SB], F32, tag="sc",
                                name="ps_qk")[:, 0, :]
            else:
                ps = mm_ps.tile([128, SB], F32, tag="mm", name="ps_qk")
            for dc in range(NDC):
                nc.tensor.matmul(ps[:], w_sb[dc][:, ncol], xt[sb][dc][:],
                                 start=(dc == 0),
                                 stop=(dc == NDC - 1 and not with_bias))
            if with_bias:
                b_sb = bq_sb if wname == "q" else bk_sb
                nc.tensor.matmul(ps[:], b_sb[0:1, ncol], ones_row[0:1, :],
                                 start=False, stop=True)
            # rope: dst = ps*cos + shuffle(ps)*sin
            t_cos = rtmp.tile([128, SB], F32, tag="rc", name="t_cos")
            nc.vector.tensor_mul(t_cos[:], ps[:], cos_bf[:, ss])
            t_shuf = rtmp.tile([128, SB], F32, tag="rs", name="t_shuf")
            nc.vector.stream_shuffle(t_shuf[:], ps[:], SHUF_MASK)
            t_sin = rtmp.tile([128, SB], F32, tag="rm", name="t_sin")
            nc.gpsimd.tensor_mul(t_sin[:], t_shuf[:], sin_bf[:, ss])
            dst = qt[c][sb] if wname == "q" else kth[c][sb]
            while pending_add:
                pending_add.popleft()()
            pending_add.append(
                lambda dst=dst, t_cos=t_cos, t_sin=t_sin:
                    nc.vector.tensor_add(dst[:], t_cos[:], t_sin[:]))

        def emit_v_st(sb, st4, pool=None):
            """V projection for one 128-seq tile, scattered into vaug."""
            st = 4 * sb + st4
            if pool == "sc":
                ps = sc_ps.tile([128, 2, SB], F32, tag="sc",
                                name="ps_v")[:, 0, :]
            else:
                ps = mm_ps.tile([128, SB], F32, tag="mm", name="ps_v")
            for dc in range(NDC):
                nc.tensor.matmul(ps[:, 0:NCOLS],
                                 xt[sb][dc][:, 128 * st4:128 * st4 + 128],
                                 wv_sb[dc][:],
                                 start=(dc == 0),
                                 stop=(dc == NDC - 1 and not with_bias))
            if with_bias:
                nc.tensor.matmul(ps[:, 0:NCOLS], ones_row[0:1, 0:128],
                                 bv_sb[0:1, :], start=False, stop=True)
            va = vaug[st][:].rearrange("p (c w) -> p c w", c=2)
            psv = ps[:, 0:NCOLS].rearrange("p (h k) -> p h k", h=HG)
            nc.vector.tensor_copy(va[:, :, 0:DK], psv[:, 0::2, :])
            nc.vector.tensor_copy(va[:, :, P0W + DK:P0W + 2 * DK],
                                  psv[:, 1::2, :])
            nc.vector.tensor_copy(va[:, :, DK], onesc_sb[:, 0:2])
            nc.vector.tensor_copy(va[:, :, P0W], onesc_sb[:, 0:2])
            while pending_add:
                pending_add.popleft()()

        def emit_wo(st, dc, copy_eng="v", ps=None, chunks=(0, 1), half=False):
            """Output projection for one (128-seq, 512-dmodel) tile. With
            half=True only the given chunk's head pair contributes and the
            partial goes to out2 (block-3 rows; the host adds it)."""
            jb = st // 4
            rq = slice(128 * (st % 4), 128 * (st % 4) + 128)
            cols = slice(SB * dc, SB * dc + SB)
            if ps is None:
                ps = mm_ps.tile([128, SB], F32, tag="mm", name="ps_wo")
            for ci, c in enumerate(chunks):
                nc.tensor.matmul(ps[:], ot2[c][jb][:, rq], wo_sb[:, c, cols],
                                 start=(ci == 0), stop=(ci == len(chunks) - 1))
            o_sb = opool.tile([128, SB], BF16, tag="osb", name="o_sb")
            if copy_eng == "s":
                nc.scalar.copy(o_sb[:], ps[:])
            else:
                nc.vector.tensor_copy(o_sb[:], ps[:])
            if half:
                nc.sync.dma_start(out2[128 * (st % 4):128 * (st % 4) + 128, cols],
                                  o_sb[:])
            else:
                nc.sync.dma_start(out[128 * st:128 * st + 128, cols], o_sb[:])

        # --- projections for block 0 chunk 0 (pair 1's chunks become
        # the first fillers inside pair 0's attention)
        emit_qk_chunk(0, 0, "q")
        emit_qk_chunk(0, 0, "k")
        for st4 in range(4):
            emit_v_st(0, st4)

        # --- main interleaved stream ------------------------------------
        seq = os.environ.get("KERNEL_SEQ", "0") == "1"
        if seq:
            emit_qk_chunk(0, 1, "q")
            emit_qk_chunk(0, 1, "k")
            for nb in range(1, NSB):
                emit_qk_chunk(nb, 0, "q")
                emit_qk_chunk(nb, 0, "k")
                emit_qk_chunk(nb, 1, "q")
                emit_qk_chunk(nb, 1, "k")
                for st4 in range(4):
                    emit_v_st(nb, st4)
        fillers = deque()
        if not seq:
            fillers.append(lambda: emit_qk_chunk(0, 1, "q"))
            fillers.append(lambda: emit_qk_chunk(0, 1, "k"))
        pending_norm = deque()  # closures, flushed in the next 128-mode stretch
        prev_pair = None  # (emit_pv, [(g, e), (g, e)], tail?) one pair behind
        for j in range(NSB):
            if not seq and j < NSB - 1:
                nb = j + 1
                # interleave projection kinds: consecutive qk chunks would
                # serialize on the 2-buffer mm psum pool behind the RoPE
                # chain's DVE reads; a V tile between them breaks the chain
                fillers.append(lambda nb=nb: emit_qk_chunk(nb, 0, "q"))
                fillers.append(lambda nb=nb, st4=0: emit_v_st(nb, st4))
                fillers.append(lambda nb=nb: emit_qk_chunk(nb, 0, "k"))
                fillers.append(lambda nb=nb, st4=1: emit_v_st(nb, st4))
                fillers.append(lambda nb=nb: emit_qk_chunk(nb, 1, "q"))
                fillers.append(lambda nb=nb, st4=2: emit_v_st(nb, st4))
                fillers.append(lambda nb=nb: emit_qk_chunk(nb, 1, "k"))
                fillers.append(lambda nb=nb, st4=3: emit_v_st(nb, st4))
            wo_blocks = {1: [0], 3: [1, 2]}
            if not seq:
                for jb in wo_blocks.get(j, []):
                    for st in range(4 * jb, 4 * jb + 4):
                        for dc in range(2):
                            fillers.append(
                                lambda st=st, dc=dc: emit_wo(st, dc))

            # Bresenham-spread the filler pops across the whole block so
            # late steps (right before the next block / the tail) are not
            # left dry by front-loaded FIFO popping
            navail = len(fillers) + (8 if (j == 3 and not seq) else 0)
            steps_total = 2 * 2 * (j + 1)  # pair-iterations in this block
            fill_state = [0, 0]  # steps seen, pops done

            # Head-pair interleave: the even and odd head of a chunk run as
            # row-tiled concurrent score matmuls; groups are processed in
            # pairs so 64-mode (scores) and 128-mode (PV + fillers) stretches
            # alternate only once per pair. PV runs one pair behind, and the
            # lag carries ACROSS chunk/block boundaries so the PE queue head
            # is never blocked on the Scalar exp backlog at a boundary.
            for c in range(2):
                while pending_add:  # qt/kth of this block must be complete
                    pending_add.popleft()()
                pvs = [pv_ps.tile([128, SB], F32, tag="pv", name="pv")
                       for _ in range(2)]
                ngrp = 4 * (j + 1)
                npairs = ngrp // 2

                def emit_sc(i, j=j, c=c):
                    # both heads' score tiles fused in one 2-bank psum tile
                    # so a single exp (and mask) covers the step. For tiles
                    # on the causal diagonal (d >= 0) sq columns < 128*d are
                    # fully masked: skip them in the matmul and exp, and
                    # mask-multiply only the 128-wide triangular band (the
                    # band-local mask is the same for every diagonal).
                    d = i - 4 * j
                    c0 = 128 * max(d, 0)
                    sc = sc_ps.tile([128, 2, SB], F32, tag="sc", name="sc")
                    for parity in range(2):
                        rows = slice(DK * parity, DK * parity + DK)
                        nc.tensor.matmul(
                            sc[:, parity, c0:],
                            kth[c][i // 4][rows, 128 * (i % 4):128 * (i % 4) + 128],
                            qt[c][j][rows, c0:],
                            start=True, stop=True)
                    e = epool.tile([128, 2, SB], BF16, tag="e", name="e")
                    nc.scalar.activation(e[:, :, c0:], sc[:, :, c0:],
                                         mybir.ActivationFunctionType.Exp,
                                         scale=SCALE)
                    if d >= 0:
                        band = slice(c0, c0 + 128)
                        nc.vector.tensor_mul(e[:, :, band], e[:, :, band],
                                             mask_sb[:])
                    return e

                def make_pv(j, c, pvs, ngrp):
                    def emit_pv(i, e):
                        d = i - 4 * j
                        c0 = 128 * max(d, 0)
                        va = vaug[i][:].rearrange("p (cc w) -> p cc w", cc=2)
                        st_ = (i == 0)
                        sp_ = (i == ngrp - 1)
                        nc.tensor.matmul(pvs[0][0:DK + 1, c0:],
                                         va[:, c, 0:DK + 1], e[:, 0, c0:],
                                         start=st_, stop=sp_)
                        nc.tensor.matmul(pvs[1][:, c0:],
                                         va[:, c, P0W:P0W + 128], e[:, 1, c0:],
                                         start=st_, stop=sp_)
                    return emit_pv

                def emit_norm(pv0, pv1, den2b, cc, jj):
                    bc2 = mm_ps.tile([128, SB], F32, tag="mm", name="bc2")
                    nc.tensor.matmul(bc2[:], ones2_sb[:], den2b[:],
                                     start=True, stop=True)
                    rec = npool.tile([128, SB], F32, tag="rec", name="rec")
                    nc.vector.reciprocal_approx_fast(rec[:], bc2[:])
                    nc.vector.tensor_mul(ot2[cc][jj][0:DK, :],
                                         pv0[0:DK, :], rec[0:DK, :])
                    nc.vector.tensor_mul(ot2[cc][jj][DK:128, :],
                                         pv1[DK:128, :], rec[DK:128, :])

                def make_tail(j, c, pvs):
                    def tail():
                        pvc = npool.tile([128, SB], F32, tag="pvc",
                                         name="pvc")
                        # stage the two denominator rows (bf16) for the
                        # deferred normalize. DVE copies keep the Scalar
                        # queue clear for the next chunk's exps; the final
                        # chunk uses Scalar so the den and pv stagings run
                        # on parallel queues at the tail.
                        if j == NSB - 1 and c == 1:
                            nc.scalar.copy(den2s[c][0:1, :], pvs[0][DK:DK + 1, :])
                            nc.scalar.copy(den2s[c][32:33, :], pvs[1][0:1, :])
                        else:
                            nc.vector.tensor_copy(den2s[c][0:1, :],
                                                  pvs[0][DK:DK + 1, :])
                            nc.vector.tensor_copy(den2s[c][32:33, :],
                                                  pvs[1][0:1, :])
                        nc.vector.tensor_copy(pvc[0:DK, :], pvs[0][0:DK, :])
                        nc.vector.tensor_copy(pvc[DK:128, :],
                                              pvs[1][DK:128, :])
                        pending_norm.append(
                            lambda pvc=pvc, den2b=den2s[c], cc=c, jj=j:
                                emit_norm(pvc, pvc, den2b, cc, jj))
                    return tail

                emit_pv = make_pv(j, c, pvs, ngrp)
                for gp in range(npairs):
                    g0, g1 = 2 * gp, 2 * gp + 1
                    cur = (emit_pv,
                           [(g0, emit_sc(g0)), (g1, emit_sc(g1))],
                           make_tail(j, c, pvs) if gp == npairs - 1 else None)
                    if gp == 0 and prev_pair is not None:
                        # boundary: the carried pair's exps were enqueued a
                        # full iteration ago; emitting its PVs (and the den
                        # staging) before the fillers puts the den copies
                        # ahead of the fillers' DVE work
                        pfn, pairs, ptail = prev_pair
                        for g, e in pairs:
                            pfn(g, e)
                        if ptail is not None:
                            ptail()
                        prev_pair = None
                    if j == 3 and c == 1 and gp == 1 and not seq:
                        # block-3 chunk-0 ot2 is normalized now: its Wo half
                        # runs as fillers; the host adds the out2 partial
                        for st_ in range(12, 16):
                            for dc_ in range(2):
                                fillers.append(
                                    lambda st_=st_, dc_=dc_: emit_wo(
                                        st_, dc_, chunks=(0,), half=True))
                    fill_state[0] += 1
                    while (fillers and fill_state[1] * steps_total <
                           fill_state[0] * navail):
                        fillers.popleft()()
                        fill_state[1] += 1
                    while pending_add:
                        pending_add.popleft()()
                    if prev_pair is not None:
                        pfn, pairs, ptail = prev_pair
                        for g, e in pairs:
                            pfn(g, e)
                        if ptail is not None:
                            ptail()
                    prev_pair = cur
                    while pending_norm:
                        pending_norm.popleft()()

        # --- tail: flush the carried pair, then the last normalize + Wo for
        # block 3; dummy matmuls keep the PE warm (and busy) while the final
        # normalize chain drains
        pfn, pairs, ptail = prev_pair
        for g, e in pairs:
            pfn(g, e)
        ptail()
        while pending_norm:
            pending_norm.popleft()()
        # warm-keepers: pinned behind the final den staging (rhs dep) so the
        # scheduler cannot hoist them; they keep the PE at 2.4 GHz while the
        # final recip/mul chain drains
        warm2 = mm_ps.tile([128, SB], F32, tag="mm", name="warm2")
        for _ in range(12):
            nc.tensor.matmul(warm2[:], ones2_sb[:],
                             den2s[1][:], start=True, stop=True)
        while fillers:
            fillers.popleft()()
        if seq:
            for st in range(0, 12):
                for dc in range(2):
                    emit_wo(st, dc)
            for st in range(12, 16):
                for dc in range(2):
                    emit_wo(st, dc, chunks=(0,), half=True)
        engs = ["v", "s"]
        for st in range(4 * (NSB - 1), 4 * NSB):
            # the sc psum pool is free at the tail: both 512-wide halves of a
            # seq tile go into one 2-bank sc tile, evacuated by a single
            # [128, 1024] copy + one full-row DMA (the banks are contiguous,
            # same pattern the exp reads); only the c1 head pair remains
            # (c0 went to out2 as fillers)
            sct = sc_ps.tile([128, 2, SB], F32, tag="sc", name="wops")
            rq = slice(128 * (st % 4), 128 * (st % 4) + 128)
            for dc in range(2):
                cols = slice(SB * dc, SB * dc + SB)
                nc.tensor.matmul(sct[:, dc, :], ot2[1][NSB - 1][:, rq],
                                 wo_sb[:, 1, cols], start=True, stop=True)
            o_sb = opool.tile([128, 2 * SB], BF16, tag="osb2", name="o_sb2")
            if engs[st % 2] == "s":
                nc.scalar.copy(o_sb[:], sct[:])
            else:
                nc.vector.tensor_copy(o_sb[:], sct[:])
            nc.sync.dma_start(out[128 * st:128 * st + 128, :], o_sb[:])

        if debug_dump:
            dq = opool.tile([128, SB], F32, tag="dbg", name="dq")
            nc.vector.tensor_copy(dq[:], qt[0][1][:])
            nc.sync.dma_start(dbg_qt, dq[:])
            dk_ = opool.tile([128, SB], F32, tag="dbg", name="dk_")
            nc.vector.tensor_copy(dk_[:], kth[0][1][:])
            nc.sync.dma_start(dbg_kh, dk_[:])
            dv = opool.tile([128, 2 * CW], F32, tag="dbgv", name="dv")
            nc.vector.tensor_copy(dv[:], vaug[4][:])
            nc.sync.dma_start(dbg_va, dv[:])
            for jj in range(NSB):
                for cc in range(2):
                    do = opool.tile([128, SB], F32, tag="dbg", name="do")
                    nc.vector.tensor_copy(do[:], ot2[cc][jj][:])
                    nc.sync.dma_start(dbg_ot[jj, cc], do[:])

    nc.compile()
    return nc


_CACHED_NC = {}


def _get_program(with_bias=False):
    if with_bias not in _CACHED_NC:
        _CACHED_NC[with_bias] = build_program(with_bias=with_bias)
    return _CACHED_NC[with_bias]


# ---------------------------------------------------------------------------
# entry point
# ---------------------------------------------------------------------------

def kernel(x, token_position, Wq, bq, Wk, bk, Wv, bv, Wo, bo, _results=None):
    from concourse.bass_utils import run_bass_kernel_spmd

    in_maps = make_core_inputs(x, token_position, Wq, bq, Wk, bk, Wv, bv, Wo, bo)
    if _results is None:
        with_bias = any(float(np.abs(np.asarray(v)).max()) != 0.0
                        for v in (bq, bk, bv))
        nc = _get_program(with_bias=with_bias)
        res = run_bass_kernel_spmd(nc, in_maps, list(range(N_CORES)))
        _results = [res.results[i] for i in range(N_CORES)]
    bo = np.asarray(bo, dtype=np.float32)
    out = np.empty((B, S, D), dtype=np.float32)
    for b in range(B):
        acc = np.zeros((S, D), dtype=np.float32)
        for hg in range(HG):
            r = _results[HG * b + hg]
            acc = acc + np.asarray(r["out"], dtype=np.float32)
            acc[S - SB:] += np.asarray(r["out2"], dtype=np.float32)
        out[b] = acc + bo[None, :]
    return out
